# revision 70
# baseline (speedup 1.0000x reference)
"""AttentionalCopula Trainium2 kernel (v2).

Data-parallel over batch: 8 NeuronCores x 2 batch elements per core, with the
two elements processed together so shared-weight matmuls stream 512-col tiles
and the PE stays busy.

Key structure per layer:
  - keys/vals creation: f32r matmuls into [128,1536] PSUM tiles, plain evacs
    split between DVE and ACT.
  - attention: 16 (elem, head) streams, software-pipelined:
      scores (PE, f32r) -> exp (ACT, bf16 out) -> mask (DVE/Pool, bf16)
      -> transposed-AV (PE, bf16: exp stationary, vals moving) which lands the
      head output directly in [v, ha] orientation in PSUM; normalization +
      residual-add read the PSUM directly (no per-head transposes).
  - LayerNorm without ACT table swaps: 1/sqrt(var+eps) = exp(-0.5*ln(var+eps))
    so the ACT engine only ever uses the exp/ln/identity/relu table.
  - FF: 512-col matmuls over both elements at once.

Self-contained: hardcodes shapes from the problem spec.
"""
import math
import sys

import numpy as np

sys.path.insert(0, "/opt/trn_rl_repo")

import concourse.bass as bass  # noqa: E402
import concourse.bacc as bacc  # noqa: E402
import concourse.tile as tile  # noqa: E402
import concourse.mybir as mybir  # noqa: E402
from contextlib import ExitStack  # noqa: E402

F32 = mybir.dt.float32
F32R = mybir.dt.float32r
BF16 = mybir.dt.bfloat16
AF = mybir.ActivationFunctionType
ALU = mybir.AluOpType

B, D, NH, NS, NT = 16, 256, 512, 8, 32
NV = NS * NT          # 256
L, H, A = 4, 8, 64
HA = H * A            # 512
M = 512
R = 128
W = NH + NV           # 768
W2 = 2 * W            # 1536
EPS = 1e-5
SCALE = A ** -0.5
NCORES = 8
EPC = B // NCORES     # 2

_BUILD_CACHE = {}
_LN_SQRT = False  # True: ACT Sqrt (table swap); False: DVE Newton rsqrt
_NEWTON = 2       # Newton iterations for the DVE rsqrt
_DEBUG = False


def ts(i, n):
    return slice(i * n, (i + 1) * n)


def _build(use_ff_bias, use_de_bias, ln_affine, use_ds_bias, use_kv_bias):
    nc = bacc.Bacc(None, target_bir_lowering=False)

    def P(name, shape, out=False, dt=F32):
        return nc.declare_dram_parameter(name, shape, dt, isOutput=out)

    ki_d = P("kiT", (258, W2), dt=F32R)
    kvu_d = P("kvup", (L, 2, 2, HA), dt=F32R)  # [row (u/ones)][kv][ha]
    dsw_d = P("dswp", (256, HA), dt=F32R)
    kw_d = P("kwp", (L, 256, HA), dt=F32R)
    vw_d = P("vwp", (L, 256, HA), dt=F32R)

    f1_d = P("ffw1", (L, 512, M), dt=F32R)
    f2_d = P("ffw2", (L, M, M), dt=F32R)
    f3_d = P("ffw3", (L, M, HA), dt=F32R)
    dew_d = P("dewp", (512, R), dt=F32R)
    mask_d = P("maskm", (128, 128), dt=BF16)
    oh_d = P("onehot", (128, 4, R), dt=BF16)
    id_d = P("ident", (128, 128), dt=F32R)
    idb_d = P("identb", (128, 128), dt=BF16)
    wv_d = P("wv4", (128, 4))
    vone_d = P("vones", (128, 12, 8, 2), dt=BF16)
    onec_d = P("onescol", (128, 1), dt=F32R)
    if use_ds_bias:
        dsb_d = P("dsb", (2, HA), dt=F32R)  # row0 zeros, row1 = ds_b
    if use_ff_bias:
        fbc_d = P("ffbc", (L, 128, 12))
    if use_de_bias:
        debc_d = P("debc", (128, 256))
    if ln_affine:
        lnbc_d = P("lnbc", (L, 128, 4, HA))
    out_d = P("out", (1, EPC), out=True)
    if _DEBUG:
        dbg_att0_d = P("dbg_att0", (128, 4, HA), out=True)
        dbg_attT0_d = P("dbg_attT0", (128, 4, 2 * NV), out=True)
        dbg_keysT_d = P("dbg_keysT", (128, 4, W2), out=True)
        dbg_vals_d = P("dbg_vals", (128, 12, 8, 66), out=True, dt=BF16)
        dbg_exp_d = P("dbg_exp", (128, W2), out=True, dt=BF16)
        dbg_attres_d = P("dbg_attres", (128, 4, HA), out=True)
        dbg_att1_d = P("dbg_att1", (128, 4, HA), out=True)
        dbg_attL_d = P("dbg_attL", (128, 4, HA), out=True)
        dbg_lg_d = P("dbg_lg", (128, 256), out=True)

    with tile.TileContext(nc) as tc, ExitStack() as ctx:
        const = ctx.enter_context(tc.tile_pool(name="const", bufs=1))
        kv = ctx.enter_context(tc.tile_pool(name="kv", bufs=2))
        kvw = ctx.enter_context(tc.tile_pool(name="kvw", bufs=2))
        ffw = ctx.enter_context(tc.tile_pool(name="ffw", bufs=1))
        ep = ctx.enter_context(tc.tile_pool(name="ep", bufs=2))
        atp = ctx.enter_context(tc.tile_pool(name="atp", bufs=2))
        atT = ctx.enter_context(tc.tile_pool(name="atT", bufs=2))
        ffp = ctx.enter_context(tc.tile_pool(name="ffp", bufs=2))
        sm = ctx.enter_context(tc.tile_pool(name="sm", bufs=4))
        ps = ctx.enter_context(tc.tile_pool(name="ps", bufs=1, space="PSUM"))

        dma = nc.sync.dma_start

        # ---- inputs/constants ----
        ki0 = const.tile([128, W2], F32R, tag="ki0")
        dma(ki0[:], ki_d.ap()[0:128])
        ki1 = const.tile([128, W2], F32R, tag="ki1")
        dma(ki1[:], ki_d.ap()[128:256])
        kiu = const.tile([2, W2], F32R, tag="kiu")
        dma(kiu[:], ki_d.ap()[256:258])
        dsw_t = const.tile([128, 2, HA], F32R, tag="dsw")
        dma(dsw_t[:], dsw_d.ap().rearrange("(a p) n -> p a n", p=128))
        if use_ds_bias:
            dsb_t = const.tile([2, HA], F32R, tag="dsb")
            dma(dsb_t[:], dsb_d.ap())
        ident = const.tile([128, 128], F32R, tag="ident")
        dma(ident[:], id_d.ap())
        identb = const.tile([128, 128], BF16, tag="identb")
        dma(identb[:], idb_d.ap())
        maskm = const.tile([128, 128], BF16, tag="maskm")
        dma(maskm[:], mask_d.ap())
        # loss-only constants: tiles now, DMA emitted after layer-0 weights
        dew_t = const.tile([128, 4, R], F32R, tag="dew")
        onehot_t = const.tile([128, 4, R], BF16, tag="onehot")
        wv4 = const.tile([128, 4], F32, tag="wv4")
        ones_col = const.tile([128, 1], F32R, tag="onescol")
        if use_de_bias:
            debc_t = const.tile([128, 256], F32, tag="debc")
        eps_t = const.tile([128, 1], F32, tag="eps")
        nc.gpsimd.memset(eps_t[:], EPS)

        mm = nc.tensor.matmul

        # evac engine rotation: DVE / ACT
        rot = [0]

        def cp(out_ap, in_ap):
            if rot[0] % 2 == 0:
                nc.vector.tensor_copy(out_ap, in_ap)
            else:
                nc.scalar.copy(out_ap, in_ap)
            rot[0] += 1

        def relu_ev(out_ap, in_ap, bias_ap):
            if rot[0] % 2 == 0:
                if bias_ap is None:
                    nc.vector.tensor_scalar_max(out_ap, in_ap, 0.0)
                else:
                    nc.vector.tensor_scalar(out_ap, in_ap, bias_ap, 0.0,
                                            op0=ALU.add, op1=ALU.max)
            else:
                if bias_ap is None:
                    nc.scalar.activation(out_ap, in_ap, AF.Relu)
                else:
                    nc.scalar.activation(out_ap, in_ap, AF.Relu, bias=bias_ap)
            rot[0] += 1

        def ln_pre(in4):
            """bn stats + batched Newton rsqrt on DVE -> (rs4, nb4)."""
            mv4 = sm.tile([128, 2, 4], F32, tag="mv4")
            for evt in range(4):
                st6 = sm.tile([128, 6], F32, tag="st6")
                nc.vector.bn_stats(st6[:], in4[:, evt, :])
                nc.vector.bn_aggr(mv4[:, :, evt], st6[:])
            x4 = sm.tile([128, 4], F32, tag="x4")
            nc.vector.tensor_scalar(x4[:], mv4[:, 1, :], EPS, None, op0=ALU.add)
            if _LN_SQRT:
                sd4 = sm.tile([128, 4], F32, tag="sd4")
                nc.scalar.activation(sd4[:], x4[:], AF.Sqrt)
                rs4 = sm.tile([128, 4], F32, tag="rs4")
                nc.vector.reciprocal(rs4[:], sd4[:])
            else:
                I32 = mybir.dt.int32
                yi = sm.tile([128, 4], I32, tag="yi")
                nc.vector.tensor_scalar(yi[:], x4[:].bitcast(I32), 1, None,
                                        op0=ALU.arith_shift_right)
                nc.vector.tensor_scalar(yi[:], yi[:], -1, 0x5f3759df,
                                        op0=ALU.mult, op1=ALU.add)
                rs4 = yi[:].bitcast(F32)
                t4 = sm.tile([128, 4], F32, tag="t4")
                for _ in range(_NEWTON):
                    nc.vector.tensor_mul(t4[:], rs4, rs4)
                    nc.vector.tensor_mul(t4[:], t4[:], x4[:])
                    nc.vector.tensor_scalar(t4[:], t4[:], -0.5, 1.5,
                                            op0=ALU.mult, op1=ALU.add)
                    nc.vector.tensor_mul(rs4, rs4, t4[:])
            nb4 = sm.tile([128, 4], F32, tag="nb4")
            nc.vector.scalar_tensor_tensor(nb4[:], mv4[:, 0, :], -1.0, rs4,
                                           op0=ALU.mult, op1=ALU.mult)
            return rs4, nb4

        def ln_post(out4, in4, rs4, nb4, lnbc_t, which):
            """Apply (x*rs + nb) [*g + b] -- all on ACT (idle during LN)."""
            for evt in range(4):
                rs = rs4[:, evt:evt + 1]
                nb = nb4[:, evt:evt + 1]
                dst = out4[:, evt, :]
                if ln_affine:
                    tmp = sm.tile([128, HA], F32, tag="lntmp")
                    nc.scalar.activation(tmp[:], in4[:, evt, :], AF.Identity,
                                         bias=nb, scale=rs)
                    g = lnbc_t[:, which * 2, :]
                    b = lnbc_t[:, which * 2 + 1, :]
                    nc.vector.tensor_mul(tmp[:], tmp[:], g)
                    nc.vector.tensor_add(dst, tmp[:], b)
                else:
                    nc.scalar.activation(dst, in4[:, evt, :], AF.Identity,
                                         bias=nb, scale=rs)

        def transpose_4(outT, c, src4):
            """src4 [128,4,512] natural -> outT[:, c, :] = [ha-chunk c, v-cols]."""
            p_tr = ps.tile([128, 512], F32R, tag="p1")
            for evt in range(4):
                nc.tensor.transpose(p_tr[:, ts(evt, 128)],
                                    src4[:, evt, ts(c, 128)], ident[:])
            cp(outT[:, c, :], p_tr[:])

        # ================== dimension-shifting init ==================
        att = atp.tile([128, 4, HA], F32R, tag="att")
        for evt in range(4):
            e, vt = divmod(evt, 2)
            kc = e * W + NH + vt * 128
            p = ps.tile([128, 512], F32, tag="p1")
            mm(p[:], ki0[:, kc:kc + 128], dsw_t[:, 0, :], start=True, stop=False)
            last = not use_ds_bias
            mm(p[:], ki1[:, kc:kc + 128], dsw_t[:, 1, :], start=False, stop=last)
            if use_ds_bias:
                mm(p[:], kiu[0:2, kc:kc + 128], dsb_t[:, :], start=False, stop=True)
            cp(att[:, evt, :], p[:])
        attT = atT.tile([128, 4, 2 * NV], F32R, tag="attT")
        for t in range(4):
            p = ps.tile([128, 512], F32, tag="p1")
            for e in range(2):
                pc = e * W + NH
                reg = p[:, ts(e, 256)]
                mm(reg, dsw_t[:, 0, ts(t, 128)], ki0[:, pc:pc + 256],
                   start=True, stop=False)
                last = not use_ds_bias
                mm(reg, dsw_t[:, 1, ts(t, 128)], ki1[:, pc:pc + 256],
                   start=False, stop=last)
                if use_ds_bias:
                    mm(reg, dsb_t[:, ts(t, 128)], kiu[0:2, pc:pc + 256],
                       start=False, stop=True)
            cp(attT[:, t, :], p[:])
        if _DEBUG:
            dma(dbg_att0_d.ap()[:], att[:].bitcast(F32))
            dma(dbg_attT0_d.ap()[:], attT[:].bitcast(F32))

        # ================== layers ==================
        def load_kv_weights(l):
            kw_t = kvw.tile([128, 2, HA], F32R, tag="kw")
            dma(kw_t[:], kw_d.ap()[l].rearrange("(a p) n -> p a n", p=128))
            vw_t = kvw.tile([128, 2, HA], F32R, tag="vw")
            dma(vw_t[:], vw_d.ap()[l].rearrange("(a p) n -> p a n", p=128))
            kvu_t = kvw.tile([2, 2, HA], F32R, tag="kvu")
            dma(kvu_t[:], kvu_d.ap()[l])
            return (kw_t, vw_t, kvu_t)

        nk = 2 if use_kv_bias else 1

        def emit_keys(kvt):
            """keysT[ha, w] for both elems; u/bias rows via k<=2 matmul."""
            kw_t, _, kvu_t = kvt
            keysT = kv.tile([128, 4, W2], F32R, tag="keysT")
            for t in range(4):
                p = ps.tile([128, W2], F32, tag="sc")
                for c in range(3):
                    reg = p[:, ts(c, 512)]
                    mm(reg, kw_t[:, 0, ts(t, 128)], ki0[:, ts(c, 512)],
                       start=True, stop=False)
                    mm(reg, kw_t[:, 1, ts(t, 128)], ki1[:, ts(c, 512)],
                       start=False, stop=False)
                    mm(reg, kvu_t[0:nk, 0, ts(t, 128)], kiu[0:nk, ts(c, 512)],
                       start=False, stop=True)
                cp(keysT[:, t, :], p[:])
            return keysT

        def emit_vals(kvt):
            """vals[w, (h,a)] bf16 for both elems."""
            _, vw_t, kvu_t = kvt
            vals = kv.tile([128, 12, 8, 66], BF16, tag="vals")
            dma(vals[:, :, :, 64:66], vone_d.ap())
            for g in range(4):
                p = ps.tile([128, W2], F32, tag="sc")
                for c in range(3):
                    ew = g * 3 + c
                    wlo = (ew // 6) * W + (ew % 6) * 128
                    reg = p[:, ts(c, 512)]
                    mm(reg, ki0[:, wlo:wlo + 128], vw_t[:, 0, :],
                       start=True, stop=False)
                    mm(reg, ki1[:, wlo:wlo + 128], vw_t[:, 1, :],
                       start=False, stop=False)
                    mm(reg, kiu[0:nk, wlo:wlo + 128], kvu_t[0:nk, 1, :],
                       start=False, stop=True)
                cp(vals[:, g * 3:(g + 1) * 3, :, 0:64],
                   p[:].rearrange("p (c h a) -> p c h a", c=3, h=8))
            return vals

        kvt = load_kv_weights(0)
        # loss-only const DMAs, after layer-0 weights in the queue
        dma(dew_t[:], dew_d.ap().rearrange("(a p) n -> p a n", p=128))
        dma(onehot_t[:], oh_d.ap())
        dma(wv4[:], wv_d.ap())
        dma(ones_col[:], onec_d.ap())
        if use_de_bias:
            dma(debc_t[:], debc_d.ap())

        keysT = emit_keys(kvt)
        vals = emit_vals(kvt)

        for l in range(L):
            f1_t = ffw.tile([128, 4, M], F32R, tag="f1")
            dma(f1_t[:], f1_d.ap()[l].rearrange("(a p) n -> p a n", p=128))
            f2_t = ffw.tile([128, 4, M], F32R, tag="f2")
            dma(f2_t[:], f2_d.ap()[l].rearrange("(a p) n -> p a n", p=128))
            f3_t = ffw.tile([128, 4, HA], F32R, tag="f3")
            dma(f3_t[:], f3_d.ap()[l].rearrange("(a p) n -> p a n", p=128))
            if use_ff_bias:
                fbc_t = ffw.tile([128, 12], F32, tag="fbc")
                dma(fbc_t[:], fbc_d.ap()[l])
            if ln_affine:
                lnbc_t = ffw.tile([128, 4, HA], F32, tag="lnbc")
                dma(lnbc_t[:], lnbc_d.ap()[l])
            else:
                lnbc_t = None
            if l + 1 < L:
                kvt = load_kv_weights(l + 1)

            # ---- attention: 16 (head, elem) streams, pipelined ----
            att_res = atp.tile([128, 4, HA], F32R, tag="att")

            def finish_stream(exh):
                ex, h, e = exh
                p_av = ps.tile([128, 512], F32, tag="p1")
                for vc in range(2):
                    nw = 5 if vc == 0 else 6
                    reg = p_av[:, vc * 256:vc * 256 + 66]
                    for wt in range(nw):
                        mm(reg, ex[:, wt * 256 + vc * 128: wt * 256 + (vc + 1) * 128],
                           vals[:, e * 6 + wt, h, :],
                           start=(wt == 0), stop=(wt == nw - 1))
                rec = sm.tile([128, 2], F32, tag="rec")
                nc.vector.reciprocal(
                    rec[:], p_av[:].rearrange("p (vc x) -> p vc x", vc=2)[:, :, 64])
                for vc in range(2):
                    evt = e * 2 + vc
                    nc.vector.scalar_tensor_tensor(
                        att_res[:, evt, ts(h, 64)],
                        p_av[:, vc * 256:vc * 256 + 64],
                        rec[:, vc:vc + 1],
                        att[:, evt, ts(h, 64)],
                        op0=ALU.mult, op1=ALU.add)

            prev = None
            for i in range(16):
                h, e = i // 2, i % 2
                t, base = h // 2, (h % 2) * 64
                p_sc = ps.tile([128, W2], F32, tag="sc")
                for wt in range(6):
                    mm(p_sc[:, ts(wt, 256)],
                       keysT[base:base + 64, t, e * W + wt * 128:e * W + (wt + 1) * 128],
                       attT[base:base + 64, t, ts(e, 256)],
                       start=True, stop=True)
                ex = ep.tile([128, W2], BF16, tag="exp")
                nc.scalar.activation(ex[:], p_sc[:], AF.Exp, scale=SCALE)
                mask_eng = nc.vector if i % 2 == 0 else nc.gpsimd
                mask_eng.tensor_mul(ex[:, 1024:1152], ex[:, 1024:1152], maskm[:])
                mask_eng.tensor_mul(ex[:, 1408:1536], ex[:, 1408:1536], maskm[:])
                if prev is not None:
                    finish_stream(prev)
                if _DEBUG and l == 0 and i == 0:
                    dma(dbg_exp_d.ap()[:], ex[:])
                prev = (ex, h, e)
            finish_stream(prev)
            if _DEBUG and l == 0:
                dma(dbg_keysT_d.ap()[:], keysT[:].bitcast(F32))
                dma(dbg_vals_d.ap()[:], vals[:])
                dma(dbg_attres_d.ap()[:], att_res[:].bitcast(F32))

            # ---- LN1 (next-layer keys matmuls fill the PE window) ----
            rs4, nb4 = ln_pre(att_res)
            att1 = atp.tile([128, 4, HA], F32R, tag="att")
            ln_post(att1, att_res, rs4, nb4, lnbc_t, 0)
            if l + 1 < L:
                keysT_next = emit_keys(kvt)
            if _DEBUG and l == 0:
                dma(dbg_att1_d.ap()[:], att1[:].bitcast(F32))
            att1T = atT.tile([128, 4, 2 * NV], F32R, tag="attT")
            for c in range(4):
                transpose_4(att1T, c, att1)

            # ---- FF ----
            ff1T = ffp.tile([128, 4, 512], F32R, tag="ffT")
            for mt in range(4):
                p = ps.tile([128, 512], F32, tag="p1")
                for c in range(4):
                    mm(p[:], f1_t[:, c, ts(mt, 128)], att1T[:, c, :],
                       start=(c == 0), stop=(c == 3))
                bias = fbc_t[:, mt:mt + 1] if use_ff_bias else None
                relu_ev(ff1T[:, mt, :], p[:], bias)
            ff2T = ffp.tile([128, 4, 512], F32R, tag="ffT")
            for mt in range(4):
                p = ps.tile([128, 512], F32, tag="p1")
                for c in range(4):
                    mm(p[:], f2_t[:, c, ts(mt, 128)], ff1T[:, c, :],
                       start=(c == 0), stop=(c == 3))
                bias = fbc_t[:, 4 + mt:5 + mt] if use_ff_bias else None
                relu_ev(ff2T[:, mt, :], p[:], bias)
            f3s = []
            for c in range(4):
                p = ps.tile([128, 512], F32, tag="p1")
                for k in range(4):
                    mm(p[:], f3_t[:, k, ts(c, 128)], ff2T[:, k, :],
                       start=(k == 0), stop=(k == 3))
                f3c = ffp.tile([128, 512], BF16, tag="f3s", bufs=4)
                if use_ff_bias:
                    nc.vector.tensor_scalar(f3c[:], p[:], fbc_t[:, 8 + c:9 + c],
                                            None, op0=ALU.add)
                else:
                    cp(f3c[:], p[:])
                f3s.append(f3c)
            att2_res = atp.tile([128, 4, HA], F32R, tag="att")
            for evt in range(4):
                p_tr = ps.tile([128, 512], BF16, tag="p1")
                for c in range(4):
                    nc.tensor.transpose(p_tr[:, ts(c, 128)],
                                        f3s[c][:, ts(evt, 128)], identb[:])
                nc.vector.tensor_add(att2_res[:, evt, :], p_tr[:],
                                     att1[:, evt, :])

            # ---- LN2 (next-layer vals matmuls fill the PE window) ----
            rs4, nb4 = ln_pre(att2_res)
            att2 = atp.tile([128, 4, HA], F32R, tag="att")
            ln_post(att2, att2_res, rs4, nb4, lnbc_t, 1)
            if l + 1 < L:
                vals_next = emit_vals(kvt)
                keysT, vals = keysT_next, vals_next
            att2T = atT.tile([128, 4, 2 * NV], F32R, tag="attT")
            for c in range(4):
                transpose_4(att2T, c, att2)
            att, attT = att2, att2T

        # ================== loss head ==================
        if _DEBUG:
            dma(dbg_attL_d.ap()[:], att[:].bitcast(F32))
        q4 = sm.tile([128, 4], F32R, tag="q4", bufs=1)
        for evt in range(4):
            p = ps.tile([128, 512], F32, tag="p1")
            for c in range(4):
                mm(p[:, 0:R], attT[:, c, ts(evt, 128)], dew_t[:, c, :],
                   start=(c == 0), stop=(c == 3))
            if use_de_bias:
                nc.vector.tensor_add(p[:, 0:256], p[:, 0:256], debc_t[:])
            if _DEBUG and evt == 0:
                lgdbg = sm.tile([128, 256], F32, tag="lgdbg", bufs=1)
                nc.vector.tensor_copy(lgdbg[:], p[:, 0:256])
                dma(dbg_lg_d.ap()[:], lgdbg[:])
            scr = sm.tile([128, R], F32, tag="scr", bufs=1)
            se = sm.tile([128, 1], F32, tag="se")
            nc.scalar.activation(scr[:], p[:, 0:R], AF.Exp, accum_out=se[:])
            lse = sm.tile([128, 1], F32, tag="lse")
            nc.scalar.activation(lse[:], se[:], AF.Ln)
            scr2 = sm.tile([128, R], F32, tag="scr2", bufs=1)
            pick = sm.tile([128, 1], F32, tag="pick")
            nc.vector.scalar_tensor_tensor(scr2[:], p[:, 0:R], 1.0,
                                           onehot_t[:, evt, :],
                                           op0=ALU.mult, op1=ALU.mult,
                                           accum_out=pick[:])
            nc.vector.tensor_sub(q4[:, evt:evt + 1], pick[:], lse[:])
        nc.vector.tensor_mul(q4[:], q4[:], wv4[:])
        p = ps.tile([128, 512], F32, tag="p1")
        mm(p[0:1, 0:4], ones_col[:], q4[:], start=True, stop=True)
        res_sb = sm.tile([1, EPC], F32, tag="res", bufs=1)
        fbias = -(NV - 1) * math.log(R)
        for e in range(EPC):
            tot = sm.tile([1, 1], F32, tag="tot")
            nc.vector.tensor_reduce(tot[:], p[0:1, e * 2:e * 2 + 2],
                                    mybir.AxisListType.X, ALU.add)
            nc.vector.tensor_scalar(res_sb[0:1, e:e + 1], tot[:], -1.0, fbias,
                                    op0=ALU.mult, op1=ALU.add)
        dma(out_d.ap()[0:1, :], res_sb[:])

    nc.finalize()
    return nc


def _prep_inputs(inputs):
    import ml_dtypes
    bf = ml_dtypes.bfloat16

    def f(k):
        return np.asarray(inputs[k], np.float32)

    hist, hu = f("hist_encoded"), f("hist_true_u")
    pred, pu = f("pred_encoded"), f("pred_true_u")
    key_w, key_b = f("key_w"), f("key_b")
    val_w, val_b = f("val_w"), f("val_b")
    ds_w, ds_b = f("ds_w"), f("ds_b")
    ff_w1, ff_b1 = f("ff_w1"), f("ff_b1")
    ff_w2, ff_b2 = f("ff_w2"), f("ff_b2")
    ff_w3, ff_b3 = f("ff_w3"), f("ff_b3")
    de_w, de_b = f("de_w"), f("de_b")
    ln1_g, ln1_b = f("ln1_g"), f("ln1_b")
    ln2_g, ln2_b = f("ln2_g"), f("ln2_b")

    enc = np.concatenate([hist, pred], axis=1)        # [B, W, D]
    uu = np.concatenate([hu, pu], axis=1)             # [B, W]
    kiT = np.empty((B, 258, W), np.float32)
    kiT[:, 0:256, :] = enc.transpose(0, 2, 1)
    kiT[:, 256, :] = uu
    kiT[:, 257, :] = 1.0

    kwp = key_w[:, :, 0:256, :].transpose(0, 2, 1, 3).reshape(L, 256, HA)
    vwp = val_w[:, :, 0:256, :].transpose(0, 2, 1, 3).reshape(L, 256, HA)
    # device tile kvu_t is [row (u/ones) partitions][kv][ha]
    kvup = np.empty((L, 2, 2, HA), np.float32)
    kvup[:, 0, 0, :] = key_w[:, :, 256, :].reshape(L, HA)
    kvup[:, 0, 1, :] = val_w[:, :, 256, :].reshape(L, HA)
    kvup[:, 1, 0, :] = key_b.reshape(L, HA)
    kvup[:, 1, 1, :] = val_b.reshape(L, HA)

    dewp = de_w.copy()

    rho = np.arange(128)[:, None]
    vv = np.arange(128)[None, :]
    maskm = (vv > rho).astype(bf)

    tgt = np.clip(np.floor(pu * R).astype(np.int64), 0, R - 1)  # [B, NV]
    oh_elem = np.zeros((B, 2, 128, R), np.float32)
    for vt in range(2):
        idx = tgt[:, vt * 128:(vt + 1) * 128]
        oh_elem[np.arange(B)[:, None], vt, np.arange(128)[None, :], idx] = 1.0
    oh_elem[:, 0, 0, :] = 0.0  # v=0 excluded

    wv4 = np.ones((128, 4), np.float32)
    wv4[0, 0] = 0.0
    wv4[0, 2] = 0.0

    ident = np.eye(128, dtype=np.float32)
    vones = np.zeros((128, 12, 8, 2), np.float32)
    vones[:, :, :, 0] = 1.0

    use_ff_bias = bool(np.any(ff_b1) or np.any(ff_b2) or np.any(ff_b3))
    use_de_bias = bool(np.any(de_b))
    ln_affine = bool(np.any(ln1_g != 1.0) or np.any(ln1_b) or
                     np.any(ln2_g != 1.0) or np.any(ln2_b))
    use_ds_bias = bool(np.any(ds_b))
    use_kv_bias = bool(np.any(key_b) or np.any(val_b))

    shared = {
        "dswp": ds_w, "kwp": kwp, "vwp": vwp, "kvup": kvup,
        "ffw1": ff_w1, "ffw2": ff_w2, "ffw3": ff_w3,
        "dewp": dewp, "maskm": maskm, "ident": ident,
        "identb": ident.astype(bf),
        "wv4": wv4, "vones": vones.astype(bf),
        "onescol": np.ones((128, 1), np.float32),
    }
    if use_ds_bias:
        dsb2 = np.zeros((2, HA), np.float32)
        dsb2[1] = ds_b
        shared["dsb"] = dsb2
    if use_ff_bias:
        fbc = np.empty((L, 128, 12), np.float32)
        for j, bb in enumerate((ff_b1, ff_b2, ff_b3)):
            fbc[:, :, j * 4:(j + 1) * 4] = bb.reshape(L, 4, 128).transpose(0, 2, 1)
        shared["ffbc"] = fbc
    if use_de_bias:
        debc = np.zeros((128, 256), np.float32)
        debc[:, 0:R] = de_b[None, :].repeat(128, 0)
        shared["debc"] = debc
    if ln_affine:
        lnbc = np.stack([ln1_g, ln1_b, ln2_g, ln2_b], axis=1)  # [L,4,HA]
        # device tile is [128, 4, HA]
        shared["lnbc"] = np.repeat(lnbc[:, None, :, :], 128, axis=1)

    in_maps = []
    for c in range(NCORES):
        m = dict(shared)
        kic = np.empty((258, W2), np.float32)
        ohc = np.empty((128, 4, R), np.float32)  # cast to bf16 below
        for e in range(EPC):
            be = c * EPC + e
            kic[:, e * W:(e + 1) * W] = kiT[be]
            ohc[:, e * 2:(e + 1) * 2, :] = oh_elem[be].swapaxes(0, 1)
        m["kiT"] = kic
        m["onehot"] = ohc.astype(bf)
        in_maps.append(m)
    flags = (use_ff_bias, use_de_bias, ln_affine, use_ds_bias, use_kv_bias)
    return in_maps, flags


def _get_nc(flags):
    if flags not in _BUILD_CACHE:
        _BUILD_CACHE[flags] = _build(*flags)
    return _BUILD_CACHE[flags]


def _run(inputs, trace=False):
    from concourse.bass_utils import run_bass_kernel_spmd
    in_maps, flags = _prep_inputs(inputs)
    nc = _get_nc(flags)
    res = run_bass_kernel_spmd(nc, in_maps, list(range(NCORES)), trace=trace)
    out = np.concatenate([res.results[c]["out"].reshape(EPC)
                          for c in range(NCORES)])
    return out.astype(np.float32), res


def kernel(**inputs) -> np.ndarray:
    out, _ = _run(inputs, trace=False)
    return out


# revision 71
# speedup vs baseline: 1.3110x; 1.3110x over previous
"""AttentionalCopula Trainium2 kernel (v2).

Data-parallel over batch: 8 NeuronCores x 2 batch elements per core, with the
two elements processed together so shared-weight matmuls stream 512-col tiles
and the PE stays busy.

Key structure per layer:
  - keys/vals creation: f32r matmuls into [128,1536] PSUM tiles, plain evacs
    split between DVE and ACT.
  - attention: 16 (elem, head) streams, software-pipelined:
      scores (PE, f32r) -> exp (ACT, bf16 out) -> mask (DVE/Pool, bf16)
      -> transposed-AV (PE, bf16: exp stationary, vals moving) which lands the
      head output directly in [v, ha] orientation in PSUM; normalization +
      residual-add read the PSUM directly (no per-head transposes).
  - LayerNorm without ACT table swaps: 1/sqrt(var+eps) = exp(-0.5*ln(var+eps))
    so the ACT engine only ever uses the exp/ln/identity/relu table.
  - FF: 512-col matmuls over both elements at once.

Self-contained: hardcodes shapes from the problem spec.
"""
import math
import sys

import numpy as np

sys.path.insert(0, "/opt/trn_rl_repo")

import concourse.bass as bass  # noqa: E402
import concourse.bacc as bacc  # noqa: E402
import concourse.tile as tile  # noqa: E402
import concourse.mybir as mybir  # noqa: E402
from contextlib import ExitStack  # noqa: E402

F32 = mybir.dt.float32
F32R = mybir.dt.float32r
BF16 = mybir.dt.bfloat16
AF = mybir.ActivationFunctionType
ALU = mybir.AluOpType

B, D, NH, NS, NT = 16, 256, 512, 8, 32
NV = NS * NT          # 256
L, H, A = 4, 8, 64
HA = H * A            # 512
M = 512
R = 128
W = NH + NV           # 768
W2 = 2 * W            # 1536
EPS = 1e-5
SCALE = A ** -0.5
NCORES = 8
EPC = B // NCORES     # 2

_BUILD_CACHE = {}
_LN_SQRT = False  # True: ACT Sqrt (table swap); False: DVE Newton rsqrt
_NEWTON = 1       # Newton iterations for the DVE rsqrt
_DEBUG = False


def ts(i, n):
    return slice(i * n, (i + 1) * n)


def _build(use_ff_bias, use_de_bias, ln_affine, use_ds_bias, use_kv_bias):
    nc = bacc.Bacc(None, target_bir_lowering=False)

    def P(name, shape, out=False, dt=F32):
        return nc.declare_dram_parameter(name, shape, dt, isOutput=out)

    ki_d = P("kiT", (258, W2), dt=F32R)
    kvu_d = P("kvup", (L, 2, 2, HA), dt=F32R)  # [row (u/ones)][kv][ha]
    dsw_d = P("dswp", (256, HA), dt=F32R)
    kw_d = P("kwp", (L, 256, HA), dt=F32R)
    vw_d = P("vwp", (L, 256, HA), dt=F32R)

    f1_d = P("ffw1", (L, 512, M), dt=F32R)
    f2_d = P("ffw2", (L, M, M), dt=F32R)
    f3_d = P("ffw3", (L, M, HA), dt=F32R)
    dew_d = P("dewp", (512, R), dt=F32R)
    mask_d = P("maskm", (128, 128), dt=BF16)
    oh_d = P("onehot", (128, 4, R), dt=BF16)
    id_d = P("ident", (128, 128), dt=F32R)
    idb_d = P("identb", (128, 128), dt=BF16)
    wv_d = P("wv4", (128, 4))
    vone_d = P("vones", (128, 12, 8, 2), dt=BF16)
    onec_d = P("onescol", (128, 1), dt=F32R)
    if use_ds_bias:
        dsb_d = P("dsb", (2, HA), dt=F32R)  # row0 zeros, row1 = ds_b
    if use_ff_bias:
        fbc_d = P("ffbc", (L, 128, 12))
    if use_de_bias:
        debc_d = P("debc", (128, 256))
    if ln_affine:
        lnbc_d = P("lnbc", (L, 128, 4, HA))
    out_d = P("out", (1, EPC), out=True)
    if _DEBUG:
        dbg_att0_d = P("dbg_att0", (128, 4, HA), out=True)
        dbg_attT0_d = P("dbg_attT0", (128, 4, 2 * NV), out=True)
        dbg_keysT_d = P("dbg_keysT", (128, 4, W2), out=True)
        dbg_vals_d = P("dbg_vals", (128, 12, 8, 66), out=True, dt=BF16)
        dbg_exp_d = P("dbg_exp", (128, W2), out=True, dt=BF16)
        dbg_attres_d = P("dbg_attres", (128, 4, HA), out=True)
        dbg_att1_d = P("dbg_att1", (128, 4, HA), out=True)
        dbg_attL_d = P("dbg_attL", (128, 4, HA), out=True)
        dbg_lg_d = P("dbg_lg", (128, 256), out=True)

    with tile.TileContext(nc) as tc, ExitStack() as ctx:
        const = ctx.enter_context(tc.tile_pool(name="const", bufs=1))
        kv = ctx.enter_context(tc.tile_pool(name="kv", bufs=2))
        kvw = ctx.enter_context(tc.tile_pool(name="kvw", bufs=2))
        ffw = ctx.enter_context(tc.tile_pool(name="ffw", bufs=1))
        ep = ctx.enter_context(tc.tile_pool(name="ep", bufs=2))
        atp = ctx.enter_context(tc.tile_pool(name="atp", bufs=2))
        atT = ctx.enter_context(tc.tile_pool(name="atT", bufs=2))
        ffp = ctx.enter_context(tc.tile_pool(name="ffp", bufs=2))
        sm = ctx.enter_context(tc.tile_pool(name="sm", bufs=4))
        ps = ctx.enter_context(tc.tile_pool(name="ps", bufs=1, space="PSUM"))

        dma = nc.sync.dma_start

        # ---- inputs/constants ----
        ki0 = const.tile([128, W2], F32R, tag="ki0")
        dma(ki0[:], ki_d.ap()[0:128])
        ki1 = const.tile([128, W2], F32R, tag="ki1")
        dma(ki1[:], ki_d.ap()[128:256])
        kiu = const.tile([2, W2], F32R, tag="kiu")
        dma(kiu[:], ki_d.ap()[256:258])
        dsw_t = const.tile([128, 2, HA], F32R, tag="dsw")
        dma(dsw_t[:], dsw_d.ap().rearrange("(a p) n -> p a n", p=128))
        if use_ds_bias:
            dsb_t = const.tile([2, HA], F32R, tag="dsb")
            dma(dsb_t[:], dsb_d.ap())
        ident = const.tile([128, 128], F32R, tag="ident")
        dma(ident[:], id_d.ap())
        identb = const.tile([128, 128], BF16, tag="identb")
        dma(identb[:], idb_d.ap())
        maskm = const.tile([128, 128], BF16, tag="maskm")
        dma(maskm[:], mask_d.ap())
        # loss-only constants: tiles now, DMA emitted after layer-0 weights
        dew_t = const.tile([128, 4, R], F32R, tag="dew")
        onehot_t = const.tile([128, 4, R], BF16, tag="onehot")
        wv4 = const.tile([128, 4], F32, tag="wv4")
        ones_col = const.tile([128, 1], F32R, tag="onescol")
        if use_de_bias:
            debc_t = const.tile([128, 256], F32, tag="debc")
        eps_t = const.tile([128, 1], F32, tag="eps")
        nc.gpsimd.memset(eps_t[:], EPS)

        mm = nc.tensor.matmul

        # evac engine rotation: DVE / ACT
        rot = [0]

        def cp(out_ap, in_ap):
            if rot[0] % 2 == 0:
                nc.vector.tensor_copy(out_ap, in_ap)
            else:
                nc.scalar.copy(out_ap, in_ap)
            rot[0] += 1

        def relu_ev(out_ap, in_ap, bias_ap):
            if rot[0] % 2 == 0:
                if bias_ap is None:
                    nc.vector.tensor_scalar_max(out_ap, in_ap, 0.0)
                else:
                    nc.vector.tensor_scalar(out_ap, in_ap, bias_ap, 0.0,
                                            op0=ALU.add, op1=ALU.max)
            else:
                if bias_ap is None:
                    nc.scalar.activation(out_ap, in_ap, AF.Relu)
                else:
                    nc.scalar.activation(out_ap, in_ap, AF.Relu, bias=bias_ap)
            rot[0] += 1

        def ln_pre(in4):
            """bn stats + batched Newton rsqrt on DVE -> (rs4, nb4)."""
            mv4 = sm.tile([128, 2, 4], F32, tag="mv4")
            for evt in range(4):
                st6 = sm.tile([128, 6], F32, tag="st6")
                nc.vector.bn_stats(st6[:], in4[:, evt, :])
                nc.vector.bn_aggr(mv4[:, :, evt], st6[:])
            x4 = sm.tile([128, 4], F32, tag="x4")
            nc.vector.tensor_scalar(x4[:], mv4[:, 1, :], EPS, None, op0=ALU.add)
            if _LN_SQRT:
                sd4 = sm.tile([128, 4], F32, tag="sd4")
                nc.scalar.activation(sd4[:], x4[:], AF.Sqrt)
                rs4 = sm.tile([128, 4], F32, tag="rs4")
                nc.vector.reciprocal(rs4[:], sd4[:])
            else:
                I32 = mybir.dt.int32
                yi = sm.tile([128, 4], I32, tag="yi")
                nc.vector.tensor_scalar(yi[:], x4[:].bitcast(I32), 1, None,
                                        op0=ALU.arith_shift_right)
                nc.vector.tensor_scalar(yi[:], yi[:], -1, 0x5f3759df,
                                        op0=ALU.mult, op1=ALU.add)
                rs4 = yi[:].bitcast(F32)
                t4 = sm.tile([128, 4], F32, tag="t4")
                for _ in range(_NEWTON):
                    nc.vector.tensor_mul(t4[:], rs4, rs4)
                    nc.vector.tensor_mul(t4[:], t4[:], x4[:])
                    nc.vector.tensor_scalar(t4[:], t4[:], -0.5, 1.5,
                                            op0=ALU.mult, op1=ALU.add)
                    nc.vector.tensor_mul(rs4, rs4, t4[:])
            nb4 = sm.tile([128, 4], F32, tag="nb4")
            nc.vector.scalar_tensor_tensor(nb4[:], mv4[:, 0, :], -1.0, rs4,
                                           op0=ALU.mult, op1=ALU.mult)
            return rs4, nb4

        def ln_post(out4, in4, rs4, nb4, lnbc_t, which):
            """Apply (x*rs + nb) [*g + b] -- all on ACT (idle during LN)."""
            for evt in range(4):
                rs = rs4[:, evt:evt + 1]
                nb = nb4[:, evt:evt + 1]
                dst = out4[:, evt, :]
                if ln_affine:
                    tmp = sm.tile([128, HA], F32, tag="lntmp")
                    nc.scalar.activation(tmp[:], in4[:, evt, :], AF.Identity,
                                         bias=nb, scale=rs)
                    g = lnbc_t[:, which * 2, :]
                    b = lnbc_t[:, which * 2 + 1, :]
                    nc.vector.tensor_mul(tmp[:], tmp[:], g)
                    nc.vector.tensor_add(dst, tmp[:], b)
                else:
                    nc.scalar.activation(dst, in4[:, evt, :], AF.Identity,
                                         bias=nb, scale=rs)

        def transpose_4(outT, c, src4):
            """src4 [128,4,512] natural -> outT[:, c, :] = [ha-chunk c, v-cols]."""
            p_tr = ps.tile([128, 512], F32R, tag="p1")
            for evt in range(4):
                nc.tensor.transpose(p_tr[:, ts(evt, 128)],
                                    src4[:, evt, ts(c, 128)], ident[:])
            cp(outT[:, c, :], p_tr[:])

        # ================== dimension-shifting init ==================
        att = atp.tile([128, 4, HA], F32R, tag="att")
        for evt in range(4):
            e, vt = divmod(evt, 2)
            kc = e * W + NH + vt * 128
            p = ps.tile([128, 512], F32, tag="p1")
            mm(p[:], ki0[:, kc:kc + 128], dsw_t[:, 0, :], start=True, stop=False)
            last = not use_ds_bias
            mm(p[:], ki1[:, kc:kc + 128], dsw_t[:, 1, :], start=False, stop=last)
            if use_ds_bias:
                mm(p[:], kiu[0:2, kc:kc + 128], dsb_t[:, :], start=False, stop=True)
            cp(att[:, evt, :], p[:])
        attT = atT.tile([128, 4, 2 * NV], F32R, tag="attT")
        for t in range(4):
            p = ps.tile([128, 512], F32, tag="p1")
            for e in range(2):
                pc = e * W + NH
                reg = p[:, ts(e, 256)]
                mm(reg, dsw_t[:, 0, ts(t, 128)], ki0[:, pc:pc + 256],
                   start=True, stop=False)
                last = not use_ds_bias
                mm(reg, dsw_t[:, 1, ts(t, 128)], ki1[:, pc:pc + 256],
                   start=False, stop=last)
                if use_ds_bias:
                    mm(reg, dsb_t[:, ts(t, 128)], kiu[0:2, pc:pc + 256],
                       start=False, stop=True)
            cp(attT[:, t, :], p[:])
        if _DEBUG:
            dma(dbg_att0_d.ap()[:], att[:].bitcast(F32))
            dma(dbg_attT0_d.ap()[:], attT[:].bitcast(F32))

        # ================== layers ==================
        def load_kv_weights(l):
            kw_t = kvw.tile([128, 2, HA], F32R, tag="kw")
            dma(kw_t[:], kw_d.ap()[l].rearrange("(a p) n -> p a n", p=128))
            vw_t = kvw.tile([128, 2, HA], F32R, tag="vw")
            dma(vw_t[:], vw_d.ap()[l].rearrange("(a p) n -> p a n", p=128))
            kvu_t = kvw.tile([2, 2, HA], F32R, tag="kvu")
            dma(kvu_t[:], kvu_d.ap()[l])
            return (kw_t, vw_t, kvu_t)

        nk = 2 if use_kv_bias else 1

        def emit_keys(kvt):
            """keysT[ha, w] for both elems; u/bias rows via k<=2 matmul."""
            kw_t, _, kvu_t = kvt
            keysT = kv.tile([128, 4, W2], F32R, tag="keysT")
            for t in range(4):
                p = ps.tile([128, W2], F32, tag="sc")
                for c in range(3):
                    reg = p[:, ts(c, 512)]
                    mm(reg, kw_t[:, 0, ts(t, 128)], ki0[:, ts(c, 512)],
                       start=True, stop=False)
                    mm(reg, kw_t[:, 1, ts(t, 128)], ki1[:, ts(c, 512)],
                       start=False, stop=False)
                    mm(reg, kvu_t[0:nk, 0, ts(t, 128)], kiu[0:nk, ts(c, 512)],
                       start=False, stop=True)
                cp(keysT[:, t, :], p[:])
            return keysT

        def emit_vals(kvt):
            """vals[w, (h,a)] bf16 for both elems."""
            _, vw_t, kvu_t = kvt
            vals = kv.tile([128, 12, 8, 66], BF16, tag="vals")
            dma(vals[:, :, :, 64:66], vone_d.ap())
            for g in range(4):
                p = ps.tile([128, W2], F32, tag="sc")
                for c in range(3):
                    ew = g * 3 + c
                    wlo = (ew // 6) * W + (ew % 6) * 128
                    reg = p[:, ts(c, 512)]
                    mm(reg, ki0[:, wlo:wlo + 128], vw_t[:, 0, :],
                       start=True, stop=False)
                    mm(reg, ki1[:, wlo:wlo + 128], vw_t[:, 1, :],
                       start=False, stop=False)
                    mm(reg, kiu[0:nk, wlo:wlo + 128], kvu_t[0:nk, 1, :],
                       start=False, stop=True)
                cp(vals[:, g * 3:(g + 1) * 3, :, 0:64],
                   p[:].rearrange("p (c h a) -> p c h a", c=3, h=8))
            return vals

        kvt = load_kv_weights(0)
        # loss-only const DMAs, after layer-0 weights in the queue
        dma(dew_t[:], dew_d.ap().rearrange("(a p) n -> p a n", p=128))
        dma(onehot_t[:], oh_d.ap())
        dma(wv4[:], wv_d.ap())
        dma(ones_col[:], onec_d.ap())
        if use_de_bias:
            dma(debc_t[:], debc_d.ap())

        keysT = emit_keys(kvt)
        vals = emit_vals(kvt)

        for l in range(L):
            f1_t = ffw.tile([128, 4, M], F32R, tag="f1")
            dma(f1_t[:], f1_d.ap()[l].rearrange("(a p) n -> p a n", p=128))
            f2_t = ffw.tile([128, 4, M], F32R, tag="f2")
            dma(f2_t[:], f2_d.ap()[l].rearrange("(a p) n -> p a n", p=128))
            f3_t = ffw.tile([128, 4, HA], F32R, tag="f3")
            dma(f3_t[:], f3_d.ap()[l].rearrange("(a p) n -> p a n", p=128))
            if use_ff_bias:
                fbc_t = ffw.tile([128, 12], F32, tag="fbc")
                dma(fbc_t[:], fbc_d.ap()[l])
            if ln_affine:
                lnbc_t = ffw.tile([128, 4, HA], F32, tag="lnbc")
                dma(lnbc_t[:], lnbc_d.ap()[l])
            else:
                lnbc_t = None
            if l + 1 < L:
                kvt = load_kv_weights(l + 1)

            # ---- attention: 16 (head, elem) streams, pipelined ----
            att_res = atp.tile([128, 4, HA], F32R, tag="att")

            def finish_stream(exh):
                ex, h, e = exh
                p_av = ps.tile([128, 512], F32, tag="p1")
                for vc in range(2):
                    nw = 5 if vc == 0 else 6
                    reg = p_av[:, vc * 256:vc * 256 + 66]
                    for wt in range(nw):
                        mm(reg, ex[:, wt * 256 + vc * 128: wt * 256 + (vc + 1) * 128],
                           vals[:, e * 6 + wt, h, :],
                           start=(wt == 0), stop=(wt == nw - 1))
                rec = sm.tile([128, 2], F32, tag="rec")
                nc.vector.reciprocal(
                    rec[:], p_av[:].rearrange("p (vc x) -> p vc x", vc=2)[:, :, 64])
                for vc in range(2):
                    evt = e * 2 + vc
                    nc.vector.scalar_tensor_tensor(
                        att_res[:, evt, ts(h, 64)],
                        p_av[:, vc * 256:vc * 256 + 64],
                        rec[:, vc:vc + 1],
                        att[:, evt, ts(h, 64)],
                        op0=ALU.mult, op1=ALU.add)

            prev = None
            for i in range(16):
                h, e = i // 2, i % 2
                t, base = h // 2, (h % 2) * 64
                p_sc = ps.tile([128, W2], F32, tag="sc")
                for wt in range(6):
                    mm(p_sc[:, ts(wt, 256)],
                       keysT[base:base + 64, t, e * W + wt * 128:e * W + (wt + 1) * 128],
                       attT[base:base + 64, t, ts(e, 256)],
                       start=True, stop=True)
                ex = ep.tile([128, W2], BF16, tag="exp")
                nc.scalar.activation(ex[:], p_sc[:], AF.Exp, scale=SCALE)
                mask_eng = nc.vector if i % 2 == 0 else nc.gpsimd
                mask_eng.tensor_mul(ex[:, 1024:1152], ex[:, 1024:1152], maskm[:])
                mask_eng.tensor_mul(ex[:, 1408:1536], ex[:, 1408:1536], maskm[:])
                if prev is not None:
                    finish_stream(prev)
                if _DEBUG and l == 0 and i == 0:
                    dma(dbg_exp_d.ap()[:], ex[:])
                prev = (ex, h, e)
            finish_stream(prev)
            if _DEBUG and l == 0:
                dma(dbg_keysT_d.ap()[:], keysT[:].bitcast(F32))
                dma(dbg_vals_d.ap()[:], vals[:])
                dma(dbg_attres_d.ap()[:], att_res[:].bitcast(F32))

            # ---- LN1 ----
            rs4, nb4 = ln_pre(att_res)
            att1 = atp.tile([128, 4, HA], F32R, tag="att")
            ln_post(att1, att_res, rs4, nb4, lnbc_t, 0)
            if _DEBUG and l == 0:
                dma(dbg_att1_d.ap()[:], att1[:].bitcast(F32))
            att1T = atT.tile([128, 4, 2 * NV], F32R, tag="attT")
            for c in range(4):
                transpose_4(att1T, c, att1)

            # ---- FF ----
            ff1T = ffp.tile([128, 4, 512], F32R, tag="ffT")
            for mt in range(4):
                p = ps.tile([128, 512], F32, tag="p1")
                for c in range(4):
                    mm(p[:], f1_t[:, c, ts(mt, 128)], att1T[:, c, :],
                       start=(c == 0), stop=(c == 3))
                bias = fbc_t[:, mt:mt + 1] if use_ff_bias else None
                relu_ev(ff1T[:, mt, :], p[:], bias)
            ff2T = ffp.tile([128, 4, 512], F32R, tag="ffT")
            for mt in range(4):
                p = ps.tile([128, 512], F32, tag="p1")
                for c in range(4):
                    mm(p[:], f2_t[:, c, ts(mt, 128)], ff1T[:, c, :],
                       start=(c == 0), stop=(c == 3))
                bias = fbc_t[:, 4 + mt:5 + mt] if use_ff_bias else None
                relu_ev(ff2T[:, mt, :], p[:], bias)
            f3s = []
            for c in range(4):
                p = ps.tile([128, 512], F32, tag="p1")
                for k in range(4):
                    mm(p[:], f3_t[:, k, ts(c, 128)], ff2T[:, k, :],
                       start=(k == 0), stop=(k == 3))
                f3c = ffp.tile([128, 512], BF16, tag="f3s", bufs=4)
                if use_ff_bias:
                    nc.vector.tensor_scalar(f3c[:], p[:], fbc_t[:, 8 + c:9 + c],
                                            None, op0=ALU.add)
                else:
                    cp(f3c[:], p[:])
                f3s.append(f3c)
            att2_res = atp.tile([128, 4, HA], F32R, tag="att")
            for evt in range(4):
                p_tr = ps.tile([128, 512], BF16, tag="p1")
                for c in range(4):
                    nc.tensor.transpose(p_tr[:, ts(c, 128)],
                                        f3s[c][:, ts(evt, 128)], identb[:])
                nc.vector.tensor_add(att2_res[:, evt, :], p_tr[:],
                                     att1[:, evt, :])

            # ---- LN2 (next-layer keys/vals fill the PE window, and the
            # long matmul burst enters the next attention phase HAM-warm) ----
            rs4, nb4 = ln_pre(att2_res)
            att2 = atp.tile([128, 4, HA], F32R, tag="att")
            ln_post(att2, att2_res, rs4, nb4, lnbc_t, 1)
            if l + 1 < L:
                keysT = emit_keys(kvt)
                vals = emit_vals(kvt)
            att2T = atT.tile([128, 4, 2 * NV], F32R, tag="attT")
            for c in range(4):
                transpose_4(att2T, c, att2)
            att, attT = att2, att2T

        # ================== loss head ==================
        if _DEBUG:
            dma(dbg_attL_d.ap()[:], att[:].bitcast(F32))
        q4 = sm.tile([128, 4], F32R, tag="q4", bufs=1)
        for evt in range(4):
            p = ps.tile([128, 512], F32, tag="p1")
            for c in range(4):
                mm(p[:, 0:R], attT[:, c, ts(evt, 128)], dew_t[:, c, :],
                   start=(c == 0), stop=(c == 3))
            if use_de_bias:
                nc.vector.tensor_add(p[:, 0:256], p[:, 0:256], debc_t[:])
            if _DEBUG and evt == 0:
                lgdbg = sm.tile([128, 256], F32, tag="lgdbg", bufs=1)
                nc.vector.tensor_copy(lgdbg[:], p[:, 0:256])
                dma(dbg_lg_d.ap()[:], lgdbg[:])
            scr = sm.tile([128, R], F32, tag="scr", bufs=1)
            se = sm.tile([128, 1], F32, tag="se")
            nc.scalar.activation(scr[:], p[:, 0:R], AF.Exp, accum_out=se[:])
            lse = sm.tile([128, 1], F32, tag="lse")
            nc.scalar.activation(lse[:], se[:], AF.Ln)
            scr2 = sm.tile([128, R], F32, tag="scr2", bufs=1)
            pick = sm.tile([128, 1], F32, tag="pick")
            nc.vector.scalar_tensor_tensor(scr2[:], p[:, 0:R], 1.0,
                                           onehot_t[:, evt, :],
                                           op0=ALU.mult, op1=ALU.mult,
                                           accum_out=pick[:])
            nc.vector.tensor_sub(q4[:, evt:evt + 1], pick[:], lse[:])
        nc.vector.tensor_mul(q4[:], q4[:], wv4[:])
        p = ps.tile([128, 512], F32, tag="p1")
        mm(p[0:1, 0:4], ones_col[:], q4[:], start=True, stop=True)
        res_sb = sm.tile([1, EPC], F32, tag="res", bufs=1)
        fbias = -(NV - 1) * math.log(R)
        for e in range(EPC):
            tot = sm.tile([1, 1], F32, tag="tot")
            nc.vector.tensor_reduce(tot[:], p[0:1, e * 2:e * 2 + 2],
                                    mybir.AxisListType.X, ALU.add)
            nc.vector.tensor_scalar(res_sb[0:1, e:e + 1], tot[:], -1.0, fbias,
                                    op0=ALU.mult, op1=ALU.add)
        dma(out_d.ap()[0:1, :], res_sb[:])

    nc.finalize()
    return nc


def _prep_inputs(inputs):
    import ml_dtypes
    bf = ml_dtypes.bfloat16

    def f(k):
        return np.asarray(inputs[k], np.float32)

    hist, hu = f("hist_encoded"), f("hist_true_u")
    pred, pu = f("pred_encoded"), f("pred_true_u")
    key_w, key_b = f("key_w"), f("key_b")
    val_w, val_b = f("val_w"), f("val_b")
    ds_w, ds_b = f("ds_w"), f("ds_b")
    ff_w1, ff_b1 = f("ff_w1"), f("ff_b1")
    ff_w2, ff_b2 = f("ff_w2"), f("ff_b2")
    ff_w3, ff_b3 = f("ff_w3"), f("ff_b3")
    de_w, de_b = f("de_w"), f("de_b")
    ln1_g, ln1_b = f("ln1_g"), f("ln1_b")
    ln2_g, ln2_b = f("ln2_g"), f("ln2_b")

    enc = np.concatenate([hist, pred], axis=1)        # [B, W, D]
    uu = np.concatenate([hu, pu], axis=1)             # [B, W]
    kiT = np.empty((B, 258, W), np.float32)
    kiT[:, 0:256, :] = enc.transpose(0, 2, 1)
    kiT[:, 256, :] = uu
    kiT[:, 257, :] = 1.0

    kwp = key_w[:, :, 0:256, :].transpose(0, 2, 1, 3).reshape(L, 256, HA)
    vwp = val_w[:, :, 0:256, :].transpose(0, 2, 1, 3).reshape(L, 256, HA)
    # device tile kvu_t is [row (u/ones) partitions][kv][ha]
    kvup = np.empty((L, 2, 2, HA), np.float32)
    kvup[:, 0, 0, :] = key_w[:, :, 256, :].reshape(L, HA)
    kvup[:, 0, 1, :] = val_w[:, :, 256, :].reshape(L, HA)
    kvup[:, 1, 0, :] = key_b.reshape(L, HA)
    kvup[:, 1, 1, :] = val_b.reshape(L, HA)

    dewp = de_w.copy()

    rho = np.arange(128)[:, None]
    vv = np.arange(128)[None, :]
    maskm = (vv > rho).astype(bf)

    tgt = np.clip(np.floor(pu * R).astype(np.int64), 0, R - 1)  # [B, NV]
    oh_elem = np.zeros((B, 2, 128, R), np.float32)
    for vt in range(2):
        idx = tgt[:, vt * 128:(vt + 1) * 128]
        oh_elem[np.arange(B)[:, None], vt, np.arange(128)[None, :], idx] = 1.0
    oh_elem[:, 0, 0, :] = 0.0  # v=0 excluded

    wv4 = np.ones((128, 4), np.float32)
    wv4[0, 0] = 0.0
    wv4[0, 2] = 0.0

    ident = np.eye(128, dtype=np.float32)
    vones = np.zeros((128, 12, 8, 2), np.float32)
    vones[:, :, :, 0] = 1.0

    use_ff_bias = bool(np.any(ff_b1) or np.any(ff_b2) or np.any(ff_b3))
    use_de_bias = bool(np.any(de_b))
    ln_affine = bool(np.any(ln1_g != 1.0) or np.any(ln1_b) or
                     np.any(ln2_g != 1.0) or np.any(ln2_b))
    use_ds_bias = bool(np.any(ds_b))
    use_kv_bias = bool(np.any(key_b) or np.any(val_b))

    shared = {
        "dswp": ds_w, "kwp": kwp, "vwp": vwp, "kvup": kvup,
        "ffw1": ff_w1, "ffw2": ff_w2, "ffw3": ff_w3,
        "dewp": dewp, "maskm": maskm, "ident": ident,
        "identb": ident.astype(bf),
        "wv4": wv4, "vones": vones.astype(bf),
        "onescol": np.ones((128, 1), np.float32),
    }
    if use_ds_bias:
        dsb2 = np.zeros((2, HA), np.float32)
        dsb2[1] = ds_b
        shared["dsb"] = dsb2
    if use_ff_bias:
        fbc = np.empty((L, 128, 12), np.float32)
        for j, bb in enumerate((ff_b1, ff_b2, ff_b3)):
            fbc[:, :, j * 4:(j + 1) * 4] = bb.reshape(L, 4, 128).transpose(0, 2, 1)
        shared["ffbc"] = fbc
    if use_de_bias:
        debc = np.zeros((128, 256), np.float32)
        debc[:, 0:R] = de_b[None, :].repeat(128, 0)
        shared["debc"] = debc
    if ln_affine:
        lnbc = np.stack([ln1_g, ln1_b, ln2_g, ln2_b], axis=1)  # [L,4,HA]
        # device tile is [128, 4, HA]
        shared["lnbc"] = np.repeat(lnbc[:, None, :, :], 128, axis=1)

    in_maps = []
    for c in range(NCORES):
        m = dict(shared)
        kic = np.empty((258, W2), np.float32)
        ohc = np.empty((128, 4, R), np.float32)  # cast to bf16 below
        for e in range(EPC):
            be = c * EPC + e
            kic[:, e * W:(e + 1) * W] = kiT[be]
            ohc[:, e * 2:(e + 1) * 2, :] = oh_elem[be].swapaxes(0, 1)
        m["kiT"] = kic
        m["onehot"] = ohc.astype(bf)
        in_maps.append(m)
    flags = (use_ff_bias, use_de_bias, ln_affine, use_ds_bias, use_kv_bias)
    return in_maps, flags


def _get_nc(flags):
    if flags not in _BUILD_CACHE:
        _BUILD_CACHE[flags] = _build(*flags)
    return _BUILD_CACHE[flags]


def _run(inputs, trace=False):
    from concourse.bass_utils import run_bass_kernel_spmd
    in_maps, flags = _prep_inputs(inputs)
    nc = _get_nc(flags)
    res = run_bass_kernel_spmd(nc, in_maps, list(range(NCORES)), trace=trace)
    out = np.concatenate([res.results[c]["out"].reshape(EPC)
                          for c in range(NCORES)])
    return out.astype(np.float32), res


def kernel(**inputs) -> np.ndarray:
    out, _ = _run(inputs, trace=False)
    return out
